# revision 12
# baseline (speedup 1.0000x reference)
"""Distributed GQA attention kernel for 8 TRN2 NeuronCores.

Problem: B=2, S=2048, D=2048, H=32 heads, KVH=4 kv-heads, HD=64 (GQA),
RoPE + causal attention + output projection, fp32 inputs/outputs.

Sharding: tensor-parallel over heads. Core c owns q-heads [4c..4c+4) and
kv-head c//2 (each kv head is shared by 2 cores; its tiny K/V projection is
recomputed on both). Per core:
  1. QKV projection from the replicated, host-pre-transposed x^T (bf16) with
     the core's weight column slice packed as one [2048, 448] bf16 rhs (k duplicated so KT
     transposes land partition-replicated).
  2. RoPE in natural layout on the DVE (weight columns de-interleaved on host
     so each head is [32 reals | 32 imags]; q.k is invariant under a common
     permutation of head dims).
  3. Q,K transposed on the PE; scores are computed transposed
     (scoresT[kpos, q]) so the softmax normalizer falls out of a ones-column
     appended to V in the PV matmul.
  4. Causal flash attention in bf16, kpos chunks processed in pairs: two
     128-kpos score matmuls into one 2-bank psum, one [128,1024] exp on ACT,
     multiplicative 0/1 mask on the diagonal chunks (post-exp, bf16 DVE),
     two PV matmuls. Diagonal pairs run FIRST within each q chunk so the
     DVE mask latency hides behind the clean chunks; phase-1 transposes lag
     their RoPE by one row tile for the same reason.
  5. Normalization: fast-approx reciprocal of the sums row (DVE), broadcast
     across 64 partitions on the idle GPSIMD engine, one DVE multiply.
  6. Attention outputs staged (transposed) to DRAM in AllToAll layout; one
     AllToAll per batch so batch-0 comm overlaps batch-1 compute. at-tile
     loads issue from the gpsimd queue so no other queue ever blocks on a
     collective.
  7. Row-sharded output projection (rows [256c..256c+256) of each batch)
     against the fully-resident bf16 wo, in two phases (batch-0 rows first)
     with explicit ordering deps so the in-order PE queue never waits on the
     second AllToAll before running work that is already eligible.
Host gathers the 8 [512, 2048] row-slices into the (2, 2048, 2048) output.
"""

import os
import sys

sys.path.insert(0, "/opt/trn_rl_repo")

import ml_dtypes
import numpy as np

import concourse.bass as bass
import concourse.mybir as mybir
import concourse.tile as tile
from concourse import bacc
from concourse.bass_utils import run_bass_kernel_spmd
from concourse.masks import make_identity
from concourse.tile_rust import add_dep_helper

N_CORES = 8
TRIM = 1  # 1: trim fully-masked leading cols of diagonal chunks; -1: off
B, S, D = 2, 2048, 2048
H, KVH, HD = 32, 4, 64
HPC = H // N_CORES  # 4 q heads per core
ROWS = B * S  # 4096
RPC = S // N_CORES  # 256 output rows per core per batch

F32 = mybir.dt.float32
BF16 = mybir.dt.bfloat16
EXP = mybir.ActivationFunctionType.Exp
ADD = mybir.AluOpType.add
MULT = mybir.AluOpType.mult
DIV = mybir.AluOpType.divide


def build():
    nc = bacc.Bacc("TRN2", target_bir_lowering=False, debug=False, num_devices=N_CORES)

    xt = nc.declare_dram_parameter("xt", [D, ROWS], BF16, isOutput=False)
    wqkv = nc.declare_dram_parameter("wqkv", [D, 448], BF16, isOutput=False)
    wo = nc.declare_dram_parameter("wo", [D, D], BF16, isOutput=False)
    ropec = nc.declare_dram_parameter("ropec", [S, 384], BF16, isOutput=False)
    ropes = nc.declare_dram_parameter("ropes", [S, 384], BF16, isOutput=False)
    maskm = nc.declare_dram_parameter("maskm", [128, 1024], BF16, isOutput=False)
    out = nc.declare_dram_parameter("out", [2 * RPC, D], F32, isOutput=True)

    with tile.TileContext(nc) as tc:
        with (
            tc.tile_pool(name="sb", bufs=1) as sb,
            tc.tile_pool(name="ps", bufs=1, space="PSUM") as ps,
            tc.tile_pool(name="dr", bufs=1, space="DRAM") as dr,
        ):
            # ---- constants / weights first so projection starts ASAP ----
            identf = sb.tile([128, 128], F32, tag="identf")
            make_identity(nc, identf[:])
            identb = sb.tile([128, 128], BF16, tag="identb")
            nc.vector.tensor_copy(identb[:], identf[:])
            wqkv_sb = []
            for k in range(16):
                w = sb.tile([128, 448], BF16, tag=f"wqkv{k}", name=f"wqkv_sb{k}")
                nc.sync.dma_start(out=w[:], in_=wqkv[128 * k : 128 * (k + 1), :])
                wqkv_sb.append(w)
            maskm_sb = sb.tile([128, 1024], BF16, tag="maskm")
            # rope tables fully resident in bf16 (loaded just-in-time below)
            ct_all = sb.tile([128, 16 * 384], BF16, tag="ct_all")
            st_all = sb.tile([128, 16 * 384], BF16, tag="st_all")
            # wo tiles are declared here but DMA'd one per phase-1 rowblock so
            # the 8 MB doesn't delay the startup x^T loads.
            wo_sb = [
                sb.tile([128, D], BF16, tag=f"wo{k}", name=f"wo_sb{k}")
                for k in range(16)
            ]

            # ---- persistent per-batch activation tiles ----
            qt_sb = [[None, None], [None, None]]  # [b][i]: [128, 2048] bf16
            kt_sb = [None, None]  # [b]: [128, 2048] bf16 (KT replicated 0:64/64:128)
            vones = [None, None]  # [b]: [128, 16*65] bf16 (V | ones columns)
            for b in range(B):
                for i in range(2):
                    t = sb.tile([128, S], BF16, tag=f"qt{b}{i}", name=f"qt{b}{i}")
                    qt_sb[b][i] = t
                kt_sb[b] = sb.tile([128, S], BF16, tag=f"kt{b}", name=f"kt{b}")
                v = sb.tile([128, 16 * 65], BF16, tag=f"v{b}", name=f"vones{b}")
                nc.vector.memset(v[:], 1.0)
                vones[b] = v

            # ================= phase 1: QKV projection + RoPE + transposes
            pend = []  # lagged transpose work
            for rb in range(8):  # 512-row blocks of the flattened (B*S) rows
                xts = []
                for k in range(16):
                    t = sb.tile([128, 512], BF16, tag="xt", bufs=18, name=f"xt_{rb}_{k}")
                    # first block on the (startup-idle) ACT HWDGE queue so it
                    # streams in parallel with the weight loads on Sync
                    eng = nc.scalar if rb == 0 else nc.sync
                    eng.dma_start(
                        out=t[:],
                        in_=xt[128 * k : 128 * (k + 1), 512 * rb : 512 * (rb + 1)],
                    )
                    xts.append(t)
                # pace the (phase-4) wo loads: two of its 16 row-tiles per block
                if rb == 0:
                    nc.sync.dma_start(out=maskm_sb[:], in_=maskm[:])
                for w in (2 * rb, 2 * rb + 1):
                    nc.sync.dma_start(out=wo_sb[w][:], in_=wo[128 * w : 128 * (w + 1), :])
                if rb < 4:  # rope tiles for this block's positions (b1 reuses them)
                    for kc2 in range(4 * rb, 4 * rb + 4):
                        nc.sync.dma_start(
                            out=ct_all[:, 384 * kc2 : 384 * (kc2 + 1)],
                            in_=ropec[128 * kc2 : 128 * (kc2 + 1), :],
                        )
                        nc.sync.dma_start(
                            out=st_all[:, 384 * kc2 : 384 * (kc2 + 1)],
                            in_=ropes[128 * kc2 : 128 * (kc2 + 1), :],
                        )
                for rt in range(4):
                    r = 4 * rb + rt  # global 128-row tile index (0..31)
                    b = r // 16
                    kc = r % 16  # position tile within the batch
                    pq = ps.tile([128, 448], F32, tag="pq", bufs=2, name=f"pq_{r}")
                    for k in range(16):
                        nc.tensor.matmul(
                            pq[:],
                            xts[k][:, 128 * rt : 128 * (rt + 1)],
                            wqkv_sb[k][:],
                            start=(k == 0),
                            stop=(k == 15),
                        )
                    # single psum read frees the pq slot in ~0.6us; RoPE and
                    # the V copy then work from SBUF (bf16 fast modes)
                    pqc = sb.tile([128, 448], BF16, tag="pqc", bufs=2, name=f"pc_{r}")
                    nc.vector.tensor_copy(pqc[:], pq[:])
                    # RoPE over q + the two k replicas (6 head-blocks of [32r|32i])
                    ct = ct_all[:, 384 * kc : 384 * (kc + 1)]
                    st = st_all[:, 384 * kc : 384 * (kc + 1)]
                    tmp1 = sb.tile([128, 384], BF16, tag="tmp1", bufs=2, name=f"t1_{r}")
                    tmp2 = sb.tile([128, 384], BF16, tag="tmp2", bufs=2, name=f"t2_{r}")
                    qk = sb.tile([128, 384], BF16, tag="qk", bufs=4, name=f"qk_{r}")
                    nc.vector.tensor_tensor(tmp1[:], pqc[:, 0:384], ct, op=MULT)
                    pqv = pqc[:, 0:384].rearrange("p (h s j) -> p h s j", s=2, j=32)
                    t2v = tmp2[:].rearrange("p (h s j) -> p h s j", s=2, j=32)
                    stv = st.rearrange("p (h s j) -> p h s j", s=2, j=32)
                    # out real-half = q_imag * (-sin); out imag-half = q_real * (+sin)
                    nc.vector.tensor_tensor(
                        t2v[:, :, 0, :], pqv[:, :, 1, :], stv[:, :, 0, :], op=MULT
                    )
                    nc.vector.tensor_tensor(
                        t2v[:, :, 1, :], pqv[:, :, 0, :], stv[:, :, 1, :], op=MULT
                    )
                    nc.vector.tensor_tensor(qk[:], tmp1[:], tmp2[:], op=ADD)
                    # V -> bf16 into the ones-padded PV weights
                    nc.scalar.copy(vones[b][:, 65 * kc : 65 * kc + 64], pqc[:, 384:448])
                    # PE transposes, lagged one rowtile so the RoPE chain has
                    # a full projection's lead time (k is duplicated in the
                    # projection so KT lands replicated in one shot)
                    pend.append((qk, b, kc, r))
                    todo = [pend.pop(0)] if len(pend) > 1 else []
                    if r == 31:
                        todo += [pend.pop(0)]
                    for tqk, tb, tkc, tr in todo:
                        for i in range(2):
                            tp = ps.tile(
                                [128, 128], BF16, tag="pq", bufs=2, name=f"tp_{tr}_{i}"
                            )
                            nc.tensor.transpose(
                                tp[:], tqk[:, 128 * i : 128 * (i + 1)], identb[:]
                            )
                            nc.vector.tensor_copy(
                                qt_sb[tb][i][:, 128 * tkc : 128 * (tkc + 1)], tp[:]
                            )
                        tpk = ps.tile([128, 128], BF16, tag="pq", bufs=2, name=f"tpk_{tr}")
                        nc.tensor.transpose(tpk[:], tqk[:, 256:384], identb[:])
                        nc.vector.tensor_copy(
                            kt_sb[tb][:, 128 * tkc : 128 * (tkc + 1)], tpk[:]
                        )

            # ================= phases 2+3: attention per batch, then AllToAll
            # run attention strictly after phase 1 (the overlap costs more in
            # in-order-queue stalls than it saves)
            a2a_out = [None, None]
            last_pv = None  # ordering handle for the output projection
            for b in range(B):
                a2a_in = dr.tile([2048, RPC], BF16, tag=f"a2ai{b}", name=f"a2a_in{b}")
                a2a_out[b] = dr.tile([2048, RPC], BF16, tag=f"a2ao{b}", name=f"a2a_out{b}")
                for hp in range(2):  # head pairs: heads (2hp, 2hp+1) at rows 0:64/64:128
                    qtile = qt_sb[b][hp]
                    for qc in range(4):  # 512-wide q chunks
                        jmax = 4 * qc + 3  # causal: 128-kpos chunks 0..jmax
                        ot = [
                            ps.tile([65, 512], F32, tag=f"ot{par}", bufs=1,
                                    name=f"ot_{b}_{hp}_{qc}_{par}")
                            for par in range(2)
                        ]
                        pend = []  # lag-2 software pipeline for the PV matmuls

                        def do_pv(ent, b=b, qc=qc, jmax=jmax, ot=ot):
                            j, N, pt = ent
                            res = None
                            for par in range(2):
                                res = nc.tensor.matmul(
                                    ot[par][:, 512 - N : 512],
                                    vones[b][:, 65 * j : 65 * j + 65],
                                    pt[:, 512 * par : 512 * par + N],
                                    start=(j == jmax),
                                    stop=(j == 0),
                                )
                            return res

                        for j in range(jmax, -1, -1):
                            d = j - 4 * qc
                            # diagonal chunks: only q >= kpos columns are live
                            N = 512 if TRIM < 0 else (512 - TRIM * 128 * d if d >= 0 else 512)
                            qlo = 512 * qc + (512 - N)
                            # both parities share one 2-bank psum tile: parity e
                            # at cols [0:N], parity o at [512:512+N], so one wide
                            # exp (and one mask) covers the head pair.
                            sp = ps.tile(
                                [128, 1024], F32, tag="sp", bufs=2,
                                name=f"s_{b}_{hp}_{qc}_{j}",
                            )
                            for par in range(2):  # head parity: rows 0:64 / 64:128
                                r0 = 64 * par
                                nc.tensor.matmul(
                                    sp[:, 512 * par : 512 * par + N],
                                    kt_sb[b][r0 : r0 + 64, 128 * j : 128 * (j + 1)],
                                    qtile[r0 : r0 + 64, qlo : qlo + N],
                                    start=True,
                                    stop=True,
                                )
                            pt = sb.tile(
                                [128, 1024], BF16, tag="pt", bufs=4,
                                name=f"pt_{b}_{hp}_{qc}_{j}",
                            )
                            # the [N:512] gap holds garbage (stale psum); it is
                            # never read by the PV matmuls below
                            nc.scalar.activation(
                                pt[:, 0 : 512 + N], sp[:, 0 : 512 + N], EXP, scale=0.125
                            )
                            if d >= 0:  # within-chunk causal mask (keep: q >= kpos)
                                nc.vector.tensor_tensor(
                                    pt[:, 0 : 512 + N], pt[:, 0 : 512 + N],
                                    maskm_sb[:, 0 : 512 + N], op=MULT,
                                )
                            pend.append((j, N, pt))
                            if len(pend) > 2:
                                last_pv = do_pv(pend.pop(0))
                        while pend:
                            last_pv = do_pv(pend.pop(0))
                        # normalize: rows 0:64 are V^T P, row 64 is the softmax sum
                        for par in range(2):
                            h = 2 * hp + par
                            sums = sb.tile([1, 512], F32, tag="sums", bufs=2,
                                           name=f"sm_{b}_{h}_{qc}")
                            nc.vector.tensor_copy(sums[:], ot[par][64:65, :])
                            inv = sb.tile([1, 512], F32, tag="inv", bufs=2,
                                          name=f"iv_{b}_{h}_{qc}")
                            nc.vector.reciprocal_approx_fast(inv[:], sums[:])
                            bcast = sb.tile([64, 512], F32, tag="bcast", bufs=2,
                                            name=f"bc_{b}_{h}_{qc}")
                            nc.gpsimd.partition_broadcast(bcast[:], inv[:])
                            osb = sb.tile([64, 512], BF16, tag="osb", bufs=3,
                                          name=f"o_{b}_{h}_{qc}")
                            nc.vector.tensor_tensor(osb[:], ot[par][0:64, :], bcast[:], op=MULT)
                            # stage into AllToAll layout: dest j2 rows 256*j2..+256
                            for half in range(2):
                                j2 = 2 * qc + half
                                nc.sync.dma_start(
                                    out=a2a_in[256 * j2 + 64 * h : 256 * j2 + 64 * (h + 1), :],
                                    in_=osb[:, 256 * half : 256 * (half + 1)],
                                )
                nc.gpsimd.collective_compute(
                    "AllToAll",
                    mybir.AluOpType.bypass,
                    replica_groups=[list(range(N_CORES))],
                    ins=[a2a_in[:].opt()],
                    outs=[a2a_out[b][:].opt()],
                )
                if b == 0:
                    # at-tile batch-0 halves load right after the first AllToAll
                    # (gpsimd queue, so Sync/PE never block on the collective)
                    ats = []
                    for k in range(16):
                        t = sb.tile([128, 512], BF16, tag=f"at{k}", name=f"at_{k}")
                        nc.gpsimd.dma_start(
                            out=t[:, 0:256],
                            in_=a2a_out[0][128 * k : 128 * (k + 1), :],
                        )
                        ats.append(t)
            for k in range(16):
                # split across gpsimd + scalar queues (ACT is idle by now) to
                # halve the post-AllToAll issue tail that gates oproj phase B
                eng = nc.gpsimd if k % 2 else nc.scalar
                eng.dma_start(
                    out=ats[k][:, 256:512],
                    in_=a2a_out[1][128 * k : 128 * (k + 1), :],
                )

            # ================= phase 4: output projection (my 512 rows @ wo)
            # phase A: batch-0 rows (need only AllToAll #1); phase B: batch-1.
            # Explicit deps pin the in-order PE queue to [attn b1][A][B].
            prev_phase_last = last_pv
            for rows in ([0, 1], [2, 3]):
                phase_last = None
                for n in range(4):
                    for row in rows:
                        op = ps.tile([128, 512], F32, tag="pq", bufs=2, name=f"op_{n}_{row}")
                        for k in range(16):
                            mm = nc.tensor.matmul(
                                op[:],
                                ats[k][:, 128 * row : 128 * (row + 1)],
                                wo_sb[k][:, 512 * n : 512 * (n + 1)],
                                start=(k == 0),
                                stop=(k == 15),
                            )
                            if k == 0 and prev_phase_last is not None:
                                add_dep_helper(
                                    mm.ins,
                                    prev_phase_last.ins,
                                    sync=False,
                                    reason="pin oproj phase order in PE queue",
                                )
                            phase_last = mm
                        ob = sb.tile([128, 512], F32, tag="outsb", bufs=2, name=f"ob_{n}_{row}")
                        nc.vector.tensor_copy(ob[:], op[:])
                        nc.sync.dma_start(
                            out=out[128 * row : 128 * (row + 1), 512 * n : 512 * (n + 1)],
                            in_=ob[:],
                        )
                prev_phase_last = phase_last

    nc.finalize()
    return nc


_NC_CACHE = None


def _get_nc():
    global _NC_CACHE
    if _NC_CACHE is None:
        _NC_CACHE = build()
    return _NC_CACHE


def _prep_inputs(x, freqs_cis, mask, wq, wk, wv, wo):
    """Host-side sharding / layout prep. Returns per-core input maps."""
    bf16 = ml_dtypes.bfloat16
    xt = np.ascontiguousarray(x.reshape(ROWS, D).T.astype(bf16))  # [D, B*S]
    cos = np.ascontiguousarray(freqs_cis[:, :, 0])  # [S, 32]
    sin = np.ascontiguousarray(freqs_cis[:, :, 1])
    c64 = np.concatenate([cos, cos], axis=1)  # [S, 64]
    s64 = np.concatenate([-sin, sin], axis=1)
    ropec = np.ascontiguousarray(np.tile(c64, (1, 6)).astype(bf16))  # [S, 384]
    ropes = np.ascontiguousarray(np.tile(s64, (1, 6)).astype(bf16))
    # diagonal-chunk 0/1 keep-mask, shared by every 128-kpos chunk after the
    # leading fully-masked columns are trimmed: keep(scoreT[r, c]) = (c >= r)
    tri = (np.arange(512)[None, :] >= np.arange(128)[:, None]).astype(bf16)
    maskm = np.ascontiguousarray(np.concatenate([tri, tri], axis=1))
    perm = np.concatenate([np.arange(0, 64, 2), np.arange(1, 64, 2)])  # de-interleave
    wo_c = np.ascontiguousarray(wo.astype(bf16))

    in_maps = []
    for c in range(N_CORES):
        heads = range(HPC * c, HPC * (c + 1))
        kv = c // 2
        wq_c = np.concatenate([wq[:, 64 * h + perm] for h in heads], axis=1)
        wk_c = wk[:, 64 * kv + perm]
        wv_c = wv[:, 64 * kv : 64 * (kv + 1)]
        wqkv_c = np.ascontiguousarray(
            np.concatenate([wq_c, wk_c, wk_c, wv_c], axis=1).astype(bf16)
        )
        in_maps.append(
            {
                "xt": xt,
                "wqkv": wqkv_c,
                "wo": wo_c,
                "ropec": ropec,
                "ropes": ropes,
                "maskm": maskm,
            }
        )
    return in_maps


def kernel(x, freqs_cis, mask, wq, wk, wv, wo, _trace=False, _trace_kwargs=None):
    nc = _get_nc()
    in_maps = _prep_inputs(
        np.asarray(x, np.float32),
        np.asarray(freqs_cis, np.float32),
        np.asarray(mask, np.float32),
        np.asarray(wq, np.float32),
        np.asarray(wk, np.float32),
        np.asarray(wv, np.float32),
        np.asarray(wo, np.float32),
    )
    kwargs = {}
    if _trace:
        kwargs["trace"] = True
        if _trace_kwargs:
            kwargs.update(_trace_kwargs)
    res = run_bass_kernel_spmd(nc, in_maps, core_ids=list(range(N_CORES)), **kwargs)
    full = np.empty((B, S, D), np.float32)
    for c in range(N_CORES):
        oc = res.results[c]["out"]
        full[0, RPC * c : RPC * (c + 1)] = oc[0:RPC]
        full[1, RPC * c : RPC * (c + 1)] = oc[RPC : 2 * RPC]
    if _trace:
        kernel.last_results = res
    return full


if __name__ == "__main__":
    print("building...")
    nc = _get_nc()
    print("built")



# revision 13
# speedup vs baseline: 1.1537x; 1.1537x over previous
"""Distributed GQA attention kernel for 8 TRN2 NeuronCores.

Problem: B=2, S=2048, D=2048, H=32 heads, KVH=4 kv-heads, HD=64 (GQA),
RoPE + causal attention + output projection, fp32 inputs/outputs.

Sharding: tensor-parallel over heads. Core c owns q-heads [4c..4c+4) and
kv-head c//2 (each kv head is shared by 2 cores; its tiny K/V projection is
recomputed on both). Per core:
  1. QKV projection from the replicated, host-pre-transposed x^T (bf16) with
     the core's weight column slice packed as one [2048, 448] bf16 rhs (k
     duplicated so KT transposes land partition-replicated).
  2. RoPE in natural layout on the DVE; Q,K transposed on the PE; scores are
     computed transposed (scoresT[kpos, q]).
  3. Attention walks 128-kpos chunks; the two heads of a pair live on
     partition halves 0:64/64:128, so their score matmuls (contraction 64)
     go to distinct PE row-groups via tile_position and run concurrently.
     Both land in one 2-bank psum so a single wide exp (and mask) covers the
     pair. Diagonal chunks are trimmed to their live q columns; the ragged
     psum accumulation relies on per-element has_written semantics.
  4. The PV stationary is [V | 64 ones columns] (M=128): the matmul itself
     replicates the softmax sums across partitions 64:128, so normalization
     is 3 pure-DVE ops (no gpsimd) and never deadlocks behind a collective
     blocking the gpsimd queue.
  5. Phases are software-pipelined: attention(b0) interleaves with the QKV
     projection of b1 (PE fills ACT-bound exp gaps), attention(b1) with the
     first half of the output projection. Collectives + at-tile loads are
     the only gpsimd work.
  6. Attention outputs staged (transposed) to DRAM in AllToAll layout; one
     AllToAll per batch. Row-sharded output projection (rows 256c..256c+256
     of each batch) against the fully-resident bf16 wo.
Host gathers the 8 [512, 2048] row-slices into the (2, 2048, 2048) output.
"""

import sys

sys.path.insert(0, "/opt/trn_rl_repo")

import ml_dtypes
import numpy as np

import concourse.mybir as mybir
import concourse.tile as tile
from concourse import bacc
from concourse.bass_utils import run_bass_kernel_spmd
from concourse.masks import make_identity

N_CORES = 8
B, S, D = 2, 2048, 2048
H, KVH, HD = 32, 4, 64
HPC = H // N_CORES  # 4 q heads per core
ROWS = B * S  # 4096
RPC = S // N_CORES  # 256 output rows per core per batch

F32 = mybir.dt.float32
BF16 = mybir.dt.bfloat16
EXP = mybir.ActivationFunctionType.Exp
MULT = mybir.AluOpType.mult


def build():
    nc = bacc.Bacc("TRN2", target_bir_lowering=False, debug=False, num_devices=N_CORES)

    xt = nc.declare_dram_parameter("xt", [D, ROWS], BF16, isOutput=False)
    wqkv = nc.declare_dram_parameter("wqkv", [D, 448], BF16, isOutput=False)
    wo = nc.declare_dram_parameter("wo", [D, D], BF16, isOutput=False)
    ropec = nc.declare_dram_parameter("ropec", [S, 384], BF16, isOutput=False)
    ropes = nc.declare_dram_parameter("ropes", [S, 384], BF16, isOutput=False)
    maskm = nc.declare_dram_parameter("maskm", [128, 1024], BF16, isOutput=False)
    out = nc.declare_dram_parameter("out", [2 * RPC, D], F32, isOutput=True)

    with tile.TileContext(nc) as tc:
        with (
            tc.tile_pool(name="sb", bufs=1) as sb,
            tc.tile_pool(name="ps", bufs=1, space="PSUM") as ps,
            tc.tile_pool(name="dr", bufs=1, space="DRAM") as dr,
        ):
            # ---- constants / weights first so projection starts ASAP ----
            identf = sb.tile([128, 128], F32, tag="identf")
            make_identity(nc, identf[:])
            identb = sb.tile([128, 128], BF16, tag="identb")
            nc.vector.tensor_copy(identb[:], identf[:])
            wqkv_sb = []
            for k in range(16):
                w = sb.tile([128, 448], BF16, tag=f"wqkv{k}", name=f"wqkv_sb{k}")
                nc.sync.dma_start(out=w[:], in_=wqkv[128 * k : 128 * (k + 1), :])
                wqkv_sb.append(w)
            maskm_sb = sb.tile([128, 1024], BF16, tag="maskm")
            # rope tables fully resident in bf16 (loaded just-in-time below)
            ct_all = sb.tile([128, 16 * 384], BF16, tag="ct_all")
            st_all = sb.tile([128, 16 * 384], BF16, tag="st_all")
            # wo tiles are declared here but DMA'd two per phase-1 rowblock so
            # the 8 MB doesn't delay the startup x^T loads.
            wo_sb = [
                sb.tile([128, D], BF16, tag=f"wo{k}", name=f"wo_sb{k}")
                for k in range(16)
            ]

            # ---- persistent per-batch activation tiles ----
            qt_sb = [[None, None], [None, None]]  # [b][hp]: [128, 2048] bf16
            kt_sb = [None, None]  # [b]: [128, 2048] bf16 (KT replicated 0:64/64:128)
            vones = [None, None]  # [b]: [128, 16*128] bf16 ([V | 64 ones] per chunk)
            for b in range(B):
                for i in range(2):
                    t = sb.tile([128, S], BF16, tag=f"qt{b}{i}", name=f"qt{b}{i}")
                    qt_sb[b][i] = t
                kt_sb[b] = sb.tile([128, S], BF16, tag=f"kt{b}", name=f"kt{b}")
                v = sb.tile([128, 16 * 128], BF16, tag=f"v{b}", name=f"vones{b}")
                nc.vector.memset(v[:], 1.0)
                vones[b] = v

            # ================= QKV projection + RoPE + transposes (per batch)
            trans_pend = []  # lagged transpose work

            def do_transpose(ent):
                tqk, tb, tkc, tr = ent
                for i in range(2):
                    tp = ps.tile([128, 128], BF16, tag="pq", bufs=2, name=f"tp_{tr}_{i}")
                    nc.tensor.transpose(tp[:], tqk[:, 128 * i : 128 * (i + 1)], identb[:])
                    nc.vector.tensor_copy(
                        qt_sb[tb][i][:, 128 * tkc : 128 * (tkc + 1)], tp[:]
                    )
                tpk = ps.tile([128, 128], BF16, tag="pq", bufs=2, name=f"tpk_{tr}")
                nc.tensor.transpose(tpk[:], tqk[:, 256:384], identb[:])
                nc.vector.tensor_copy(kt_sb[tb][:, 128 * tkc : 128 * (tkc + 1)], tpk[:])

            def qkv_stream(bb):
                for rb in range(4 * bb, 4 * bb + 4):  # 512-row blocks
                    xts = []
                    for k in range(16):
                        t = sb.tile(
                            [128, 512], BF16, tag="xt", bufs=18, name=f"xt_{rb}_{k}"
                        )
                        # first block on the (startup-idle) ACT HWDGE queue so
                        # it streams in parallel with the weight loads on Sync
                        eng = nc.scalar if rb == 0 else nc.sync
                        eng.dma_start(
                            out=t[:],
                            in_=xt[128 * k : 128 * (k + 1), 512 * rb : 512 * (rb + 1)],
                        )
                        xts.append(t)
                    if rb == 0:
                        nc.sync.dma_start(out=maskm_sb[:], in_=maskm[:])
                    # pace the wo loads: two of its 16 row-tiles per block
                    for w in (2 * rb, 2 * rb + 1):
                        nc.sync.dma_start(
                            out=wo_sb[w][:], in_=wo[128 * w : 128 * (w + 1), :]
                        )
                    if rb < 4:  # rope tiles for this block's positions (b1 reuses)
                        for kc2 in range(4 * rb, 4 * rb + 4):
                            nc.sync.dma_start(
                                out=ct_all[:, 384 * kc2 : 384 * (kc2 + 1)],
                                in_=ropec[128 * kc2 : 128 * (kc2 + 1), :],
                            )
                            nc.sync.dma_start(
                                out=st_all[:, 384 * kc2 : 384 * (kc2 + 1)],
                                in_=ropes[128 * kc2 : 128 * (kc2 + 1), :],
                            )
                    for rt in range(4):
                        r = 4 * rb + rt  # global 128-row tile index (0..31)
                        b = r // 16
                        kc = r % 16  # position tile within the batch
                        pq = ps.tile([128, 448], F32, tag="pq", bufs=2, name=f"pq_{r}")
                        for k in range(16):
                            nc.tensor.matmul(
                                pq[:],
                                xts[k][:, 128 * rt : 128 * (rt + 1)],
                                wqkv_sb[k][:],
                                start=(k == 0),
                                stop=(k == 15),
                            )
                        # single psum read frees the pq slot; RoPE and the V
                        # copy then work from SBUF (bf16 fast modes)
                        pqc = sb.tile([128, 448], BF16, tag="pqc", bufs=2, name=f"pc_{r}")
                        nc.vector.tensor_copy(pqc[:], pq[:])
                        # RoPE over q + the two k replicas (6 blocks of [32r|32i])
                        ct = ct_all[:, 384 * kc : 384 * (kc + 1)]
                        st = st_all[:, 384 * kc : 384 * (kc + 1)]
                        tmp1 = sb.tile([128, 384], BF16, tag="tmp1", bufs=2, name=f"t1_{r}")
                        tmp2 = sb.tile([128, 384], BF16, tag="tmp2", bufs=2, name=f"t2_{r}")
                        qk = sb.tile([128, 384], BF16, tag="qk", bufs=4, name=f"qk_{r}")
                        nc.vector.tensor_tensor(tmp1[:], pqc[:, 0:384], ct, op=MULT)
                        pqv = pqc[:, 0:384].rearrange("p (h s j) -> p h s j", s=2, j=32)
                        t2v = tmp2[:].rearrange("p (h s j) -> p h s j", s=2, j=32)
                        stv = st.rearrange("p (h s j) -> p h s j", s=2, j=32)
                        # out real-half = q_i * (-sin); out imag-half = q_r * sin
                        nc.vector.tensor_tensor(
                            t2v[:, :, 0, :], pqv[:, :, 1, :], stv[:, :, 0, :], op=MULT
                        )
                        nc.vector.tensor_tensor(
                            t2v[:, :, 1, :], pqv[:, :, 0, :], stv[:, :, 1, :], op=MULT
                        )
                        nc.vector.tensor_tensor(qk[:], tmp1[:], tmp2[:], op=mybir.AluOpType.add)
                        # V -> bf16 into the ones-padded PV stationary
                        nc.scalar.copy(
                            vones[b][:, 128 * kc : 128 * kc + 64], pqc[:, 384:448]
                        )
                        # PE transposes, lagged one rowtile so the RoPE chain
                        # has a full projection's lead time
                        trans_pend.append((qk, b, kc, r))
                        if len(trans_pend) > 1:
                            do_transpose(trans_pend.pop(0))
                        yield
                while trans_pend:
                    do_transpose(trans_pend.pop(0))

            # ================= attention (per batch), ends with its AllToAll
            a2a_out = [None, None]

            def attn_stream(bb):
                a2a_in = dr.tile([2048, RPC], BF16, tag=f"a2ai{bb}", name=f"a2a_in{bb}")
                a2a_out[bb] = dr.tile(
                    [2048, RPC], BF16, tag=f"a2ao{bb}", name=f"a2a_out{bb}"
                )
                for hp in range(2):  # head pairs at partition halves 0:64/64:128
                    qtile = qt_sb[bb][hp]
                    for qc in range(4):  # 512-wide q chunks
                        jmax = 4 * qc + 3  # causal: 128-kpos chunks 0..jmax
                        ot = [
                            ps.tile([128, 512], F32, tag=f"ot{par}", bufs=1,
                                    name=f"ot_{bb}_{hp}_{qc}_{par}")
                            for par in range(2)
                        ]
                        pend = []  # lag-2 software pipeline for the PV matmuls

                        def do_pv(ent, bb=bb, jmax=jmax, ot=ot):
                            j, N, pt = ent
                            for par in range(2):
                                nc.tensor.matmul(
                                    ot[par][:, 512 - N : 512],
                                    vones[bb][:, 128 * j : 128 * (j + 1)],
                                    pt[:, 512 * par : 512 * par + N],
                                    start=(j == jmax),
                                    stop=(j == 0),
                                )

                        for j in range(jmax, -1, -1):
                            d = j - 4 * qc
                            # diagonal chunks: only q >= kpos columns are live
                            N = 512 - 128 * d if d >= 0 else 512
                            qlo = 512 * qc + (512 - N)
                            # both parities share one 2-bank psum tile so one
                            # wide exp (and one mask) covers the head pair
                            sp = ps.tile(
                                [128, 1024], F32, tag="sp", bufs=2,
                                name=f"s_{bb}_{hp}_{qc}_{j}",
                            )
                            for par in range(2):
                                r0 = 64 * par
                                nc.tensor.matmul(
                                    sp[:, 512 * par : 512 * par + N],
                                    kt_sb[bb][r0 : r0 + 64, 128 * j : 128 * (j + 1)],
                                    qtile[r0 : r0 + 64, qlo : qlo + N],
                                    start=True,
                                    stop=True,
                                )
                            pt = sb.tile(
                                [128, 1024], BF16, tag="pt", bufs=4,
                                name=f"pt_{bb}_{hp}_{qc}_{j}",
                            )
                            # the [N:512] gap holds garbage; never read below
                            nc.scalar.activation(
                                pt[:, 0 : 512 + N], sp[:, 0 : 512 + N], EXP, scale=0.125
                            )
                            if d >= 0:  # within-chunk causal mask (keep: q >= kpos)
                                nc.vector.tensor_tensor(
                                    pt[:, 0 : 512 + N], pt[:, 0 : 512 + N],
                                    maskm_sb[:, 0 : 512 + N], op=MULT,
                                )
                            pend.append((j, N, pt))
                            if len(pend) > 2:
                                do_pv(pend.pop(0))
                            yield
                        while pend:
                            do_pv(pend.pop(0))
                        # normalize: rows 0:64 are V^T P, rows 64:128 all hold
                        # the softmax sums (ones columns) -> pure-DVE norm
                        for par in range(2):
                            h = 2 * hp + par
                            sums = sb.tile([64, 512], F32, tag="sums", bufs=2,
                                           name=f"sm_{bb}_{h}_{qc}")
                            nc.vector.tensor_copy(sums[:], ot[par][64:128, :])
                            inv = sb.tile([64, 512], F32, tag="inv", bufs=2,
                                          name=f"iv_{bb}_{h}_{qc}")
                            nc.vector.reciprocal_approx_fast(inv[:], sums[:])
                            osb = sb.tile([64, 512], BF16, tag="osb", bufs=3,
                                          name=f"o_{bb}_{h}_{qc}")
                            nc.vector.tensor_tensor(
                                osb[:], ot[par][0:64, :], inv[:], op=MULT
                            )
                            # stage into AllToAll layout: dest j2 rows 256*j2..
                            for half in range(2):
                                j2 = 2 * qc + half
                                nc.sync.dma_start(
                                    out=a2a_in[
                                        256 * j2 + 64 * h : 256 * j2 + 64 * (h + 1), :
                                    ],
                                    in_=osb[:, 256 * half : 256 * (half + 1)],
                                )
                nc.gpsimd.collective_compute(
                    "AllToAll",
                    mybir.AluOpType.bypass,
                    replica_groups=[list(range(N_CORES))],
                    ins=[a2a_in[:].opt()],
                    outs=[a2a_out[bb][:].opt()],
                )

            # ================= output projection (my 512 rows @ wo)
            ats = [
                sb.tile([128, 512], BF16, tag=f"at{k}", name=f"at_{k}")
                for k in range(16)
            ]

            def oproj_stream(rows):
                for n in range(4):
                    for row in rows:
                        op = ps.tile([128, 512], F32, tag="pq", bufs=2,
                                     name=f"op_{n}_{row}")
                        for k in range(16):
                            nc.tensor.matmul(
                                op[:],
                                ats[k][:, 128 * row : 128 * (row + 1)],
                                wo_sb[k][:, 512 * n : 512 * (n + 1)],
                                start=(k == 0),
                                stop=(k == 15),
                            )
                        ob = sb.tile([128, 512], F32, tag="outsb", bufs=2,
                                     name=f"ob_{n}_{row}")
                        nc.vector.tensor_copy(ob[:], op[:])
                        nc.sync.dma_start(
                            out=out[128 * row : 128 * (row + 1), 512 * n : 512 * (n + 1)],
                            in_=ob[:],
                        )
                        yield

            # ---- phase 1: QKV b0
            for _ in qkv_stream(0):
                pass
            # ---- phase 2: attention b0 (ACT-bound) interleaved with QKV b1
            g_attn0, g_qkv1 = attn_stream(0), qkv_stream(1)
            cnt = 0
            for _ in g_attn0:
                cnt += 1
                if cnt % 5 == 0:
                    next(g_qkv1, None)
            for _ in g_qkv1:
                pass
            # at-tile b0 halves: gpsimd queue right behind the first AllToAll
            for k in range(16):
                nc.gpsimd.dma_start(
                    out=ats[k][:, 0:256], in_=a2a_out[0][128 * k : 128 * (k + 1), :]
                )
            # ---- phase 3: attention b1 interleaved with oproj phase A (b0 rows)
            g_attn1, g_opA = attn_stream(1), oproj_stream([0, 1])
            cnt = 0
            for _ in g_attn1:
                cnt += 1
                if cnt >= 40 and cnt % 5 == 0:
                    next(g_opA, None)
            for _ in g_opA:
                pass
            # at-tile b1 halves split across gpsimd + scalar (ACT idle by now)
            for k in range(16):
                eng = nc.gpsimd if k % 2 else nc.scalar
                eng.dma_start(
                    out=ats[k][:, 256:512], in_=a2a_out[1][128 * k : 128 * (k + 1), :]
                )
            # ---- phase 4: oproj phase B (b1 rows)
            for _ in oproj_stream([2, 3]):
                pass

    nc.finalize()
    return nc


_NC_CACHE = None


def _get_nc():
    global _NC_CACHE
    if _NC_CACHE is None:
        _NC_CACHE = build()
    return _NC_CACHE


def _prep_inputs(x, freqs_cis, mask, wq, wk, wv, wo):
    """Host-side sharding / layout prep. Returns per-core input maps."""
    bf16 = ml_dtypes.bfloat16
    xt = np.ascontiguousarray(x.reshape(ROWS, D).T.astype(bf16))  # [D, B*S]
    cos = np.ascontiguousarray(freqs_cis[:, :, 0])  # [S, 32]
    sin = np.ascontiguousarray(freqs_cis[:, :, 1])
    c64 = np.concatenate([cos, cos], axis=1)  # [S, 64]
    s64 = np.concatenate([-sin, sin], axis=1)
    ropec = np.ascontiguousarray(np.tile(c64, (1, 6)).astype(bf16))  # [S, 384]
    ropes = np.ascontiguousarray(np.tile(s64, (1, 6)).astype(bf16))
    # per-chunk 0/1 keep-mask (keep: q_local col >= kpos row), duplicated so
    # one DVE op can mask both head parities of a [e | o] pt tile
    tri = (np.arange(512)[None, :] >= np.arange(128)[:, None]).astype(bf16)
    maskm = np.ascontiguousarray(np.concatenate([tri, tri], axis=1))
    perm = np.concatenate([np.arange(0, 64, 2), np.arange(1, 64, 2)])  # de-interleave
    wo_c = np.ascontiguousarray(wo.astype(bf16))

    in_maps = []
    for c in range(N_CORES):
        heads = range(HPC * c, HPC * (c + 1))
        kv = c // 2
        wq_c = np.concatenate([wq[:, 64 * h + perm] for h in heads], axis=1)
        wk_c = wk[:, 64 * kv + perm]
        wv_c = wv[:, 64 * kv : 64 * (kv + 1)]
        wqkv_c = np.ascontiguousarray(
            np.concatenate([wq_c, wk_c, wk_c, wv_c], axis=1).astype(bf16)
        )
        in_maps.append(
            {
                "xt": xt,
                "wqkv": wqkv_c,
                "wo": wo_c,
                "ropec": ropec,
                "ropes": ropes,
                "maskm": maskm,
            }
        )
    return in_maps


def kernel(x, freqs_cis, mask, wq, wk, wv, wo, _trace=False, _trace_kwargs=None):
    nc = _get_nc()
    in_maps = _prep_inputs(
        np.asarray(x, np.float32),
        np.asarray(freqs_cis, np.float32),
        np.asarray(mask, np.float32),
        np.asarray(wq, np.float32),
        np.asarray(wk, np.float32),
        np.asarray(wv, np.float32),
        np.asarray(wo, np.float32),
    )
    kwargs = {}
    if _trace:
        kwargs["trace"] = True
        if _trace_kwargs:
            kwargs.update(_trace_kwargs)
    res = run_bass_kernel_spmd(nc, in_maps, core_ids=list(range(N_CORES)), **kwargs)
    full = np.empty((B, S, D), np.float32)
    for c in range(N_CORES):
        oc = res.results[c]["out"]
        full[0, RPC * c : RPC * (c + 1)] = oc[0:RPC]
        full[1, RPC * c : RPC * (c + 1)] = oc[RPC : 2 * RPC]
    if _trace:
        kernel.last_results = res
    return full


if __name__ == "__main__":
    print("building...")
    nc = _get_nc()
    print("built")


# revision 14
# speedup vs baseline: 1.1685x; 1.0128x over previous
"""Distributed GQA attention kernel for 8 TRN2 NeuronCores.

Problem: B=2, S=2048, D=2048, H=32 heads, KVH=4 kv-heads, HD=64 (GQA),
RoPE + causal attention + output projection, fp32 inputs/outputs.

Sharding: tensor-parallel over heads. Core c owns q-heads [4c..4c+4) and
kv-head c//2 (each kv head is shared by 2 cores; its tiny K/V projection is
recomputed on both). Per core:
  1. QKV projection from the replicated, host-pre-transposed x^T (bf16) with
     the core's weight column slice packed as one [2048, 448] bf16 rhs (k
     duplicated so KT transposes land partition-replicated).
  2. RoPE in natural layout on the DVE; Q,K transposed on the PE; scores are
     computed transposed (scoresT[kpos, q]).
  3. Attention walks 128-kpos chunks; the two heads of a pair live on
     partition halves 0:64/64:128, so their score matmuls (contraction 64)
     go to distinct PE row-groups via tile_position and run concurrently.
     Both land in one 2-bank psum so a single wide exp (and mask) covers the
     pair. Diagonal chunks are trimmed to their live q columns; the ragged
     psum accumulation relies on per-element has_written semantics.
  4. The PV stationary is [V | 64 ones columns] (M=128): the matmul itself
     replicates the softmax sums across partitions 64:128, so normalization
     is 3 pure-DVE ops (no gpsimd) and never deadlocks behind a collective
     blocking the gpsimd queue.
  5. Phases are software-pipelined: attention(b0) interleaves with the QKV
     projection of b1 (PE fills ACT-bound exp gaps), attention(b1) with the
     first half of the output projection. Collectives + at-tile loads are
     the only gpsimd work.
  6. Attention outputs staged (transposed) to DRAM in AllToAll layout; one
     AllToAll per batch. Row-sharded output projection (rows 256c..256c+256
     of each batch) against the fully-resident bf16 wo.
Host gathers the 8 [512, 2048] row-slices into the (2, 2048, 2048) output.
"""

import sys

sys.path.insert(0, "/opt/trn_rl_repo")

import ml_dtypes
import numpy as np

import concourse.mybir as mybir
import concourse.tile as tile
from concourse import bacc
from concourse.bass_utils import run_bass_kernel_spmd
from concourse.masks import make_identity

N_CORES = 8
B, S, D = 2, 2048, 2048
H, KVH, HD = 32, 4, 64
HPC = H // N_CORES  # 4 q heads per core
ROWS = B * S  # 4096
RPC = S // N_CORES  # 256 output rows per core per batch

F32 = mybir.dt.float32
BF16 = mybir.dt.bfloat16
EXP = mybir.ActivationFunctionType.Exp
MULT = mybir.AluOpType.mult


def build():
    nc = bacc.Bacc("TRN2", target_bir_lowering=False, debug=False, num_devices=N_CORES)

    xt = nc.declare_dram_parameter("xt", [D, ROWS], BF16, isOutput=False)
    wqkv = nc.declare_dram_parameter("wqkv", [D, 448], BF16, isOutput=False)
    wo = nc.declare_dram_parameter("wo", [D, D], BF16, isOutput=False)
    ropec = nc.declare_dram_parameter("ropec", [S, 384], BF16, isOutput=False)
    ropes = nc.declare_dram_parameter("ropes", [S, 384], BF16, isOutput=False)
    maskm = nc.declare_dram_parameter("maskm", [128, 1024], BF16, isOutput=False)
    out = nc.declare_dram_parameter("out", [2 * RPC, D], F32, isOutput=True)

    with tile.TileContext(nc) as tc:
        with (
            tc.tile_pool(name="sb", bufs=1) as sb,
            tc.tile_pool(name="ps", bufs=1, space="PSUM") as ps,
            tc.tile_pool(name="dr", bufs=1, space="DRAM") as dr,
        ):
            # ---- constants / weights first so projection starts ASAP ----
            identf = sb.tile([128, 128], F32, tag="identf")
            make_identity(nc, identf[:])
            identb = sb.tile([128, 128], BF16, tag="identb")
            nc.vector.tensor_copy(identb[:], identf[:])
            wqkv_sb = []
            for k in range(16):
                w = sb.tile([128, 448], BF16, tag=f"wqkv{k}", name=f"wqkv_sb{k}")
                nc.sync.dma_start(out=w[:], in_=wqkv[128 * k : 128 * (k + 1), :])
                wqkv_sb.append(w)
            maskm_sb = sb.tile([128, 1024], BF16, tag="maskm")
            # rope tables fully resident in bf16 (loaded just-in-time below)
            ct_all = sb.tile([128, 16 * 384], BF16, tag="ct_all")
            st_all = sb.tile([128, 16 * 384], BF16, tag="st_all")
            # wo tiles are declared here but DMA'd two per phase-1 rowblock so
            # the 8 MB doesn't delay the startup x^T loads.
            wo_sb = [
                sb.tile([128, D], BF16, tag=f"wo{k}", name=f"wo_sb{k}")
                for k in range(16)
            ]

            # ---- persistent per-batch activation tiles ----
            qt_sb = [[None, None], [None, None]]  # [b][hp]: [128, 2048] bf16
            kt_sb = [None, None]  # [b]: [128, 2048] bf16 (KT replicated 0:64/64:128)
            vones = [None, None]  # [b]: [128, 16*128] bf16 ([V | 64 ones] per chunk)
            for b in range(B):
                for i in range(2):
                    t = sb.tile([128, S], BF16, tag=f"qt{b}{i}", name=f"qt{b}{i}")
                    qt_sb[b][i] = t
                kt_sb[b] = sb.tile([128, S], BF16, tag=f"kt{b}", name=f"kt{b}")
                v = sb.tile([128, 16 * 128], BF16, tag=f"v{b}", name=f"vones{b}")
                nc.vector.memset(v[:], 1.0)
                vones[b] = v

            # ================= QKV projection + RoPE + transposes (per batch)
            trans_pend = []  # lagged transpose work

            def do_transpose(ent):
                tqk, tb, tkc, tr = ent
                for i in range(2):
                    tp = ps.tile([128, 128], BF16, tag="pq", bufs=2, name=f"tp_{tr}_{i}")
                    nc.tensor.transpose(tp[:], tqk[:, 128 * i : 128 * (i + 1)], identb[:])
                    nc.vector.tensor_copy(
                        qt_sb[tb][i][:, 128 * tkc : 128 * (tkc + 1)], tp[:]
                    )
                tpk = ps.tile([128, 128], BF16, tag="pq", bufs=2, name=f"tpk_{tr}")
                nc.tensor.transpose(tpk[:], tqk[:, 256:384], identb[:])
                nc.vector.tensor_copy(kt_sb[tb][:, 128 * tkc : 128 * (tkc + 1)], tpk[:])

            def qkv_stream(bb):
                for rb in range(4 * bb, 4 * bb + 4):  # 512-row blocks
                    xts = []
                    for k in range(16):
                        t = sb.tile(
                            [128, 512], BF16, tag="xt", bufs=18, name=f"xt_{rb}_{k}"
                        )
                        # first block on the (startup-idle) ACT HWDGE queue so
                        # it streams in parallel with the weight loads on Sync
                        eng = nc.scalar if rb == 0 else nc.sync
                        eng.dma_start(
                            out=t[:],
                            in_=xt[128 * k : 128 * (k + 1), 512 * rb : 512 * (rb + 1)],
                        )
                        xts.append(t)
                    if rb == 0:
                        nc.sync.dma_start(out=maskm_sb[:], in_=maskm[:])
                    # pace the wo loads: two of its 16 row-tiles per block
                    for w in (2 * rb, 2 * rb + 1):
                        nc.sync.dma_start(
                            out=wo_sb[w][:], in_=wo[128 * w : 128 * (w + 1), :]
                        )
                    if rb < 4:  # rope tiles for this block's positions (b1 reuses)
                        for kc2 in range(4 * rb, 4 * rb + 4):
                            nc.sync.dma_start(
                                out=ct_all[:, 384 * kc2 : 384 * (kc2 + 1)],
                                in_=ropec[128 * kc2 : 128 * (kc2 + 1), :],
                            )
                            nc.sync.dma_start(
                                out=st_all[:, 384 * kc2 : 384 * (kc2 + 1)],
                                in_=ropes[128 * kc2 : 128 * (kc2 + 1), :],
                            )
                    for rt in range(4):
                        r = 4 * rb + rt  # global 128-row tile index (0..31)
                        b = r // 16
                        kc = r % 16  # position tile within the batch
                        pq = ps.tile([128, 448], F32, tag="pq", bufs=2, name=f"pq_{r}")
                        for k in range(16):
                            nc.tensor.matmul(
                                pq[:],
                                xts[k][:, 128 * rt : 128 * (rt + 1)],
                                wqkv_sb[k][:],
                                start=(k == 0),
                                stop=(k == 15),
                            )
                            if k in (3, 7, 11):  # fine-grained interleave points
                                yield
                        # single psum read frees the pq slot; RoPE and the V
                        # copy then work from SBUF (bf16 fast modes)
                        pqc = sb.tile([128, 448], BF16, tag="pqc", bufs=2, name=f"pc_{r}")
                        nc.vector.tensor_copy(pqc[:], pq[:])
                        # RoPE over q + the two k replicas (6 blocks of [32r|32i])
                        ct = ct_all[:, 384 * kc : 384 * (kc + 1)]
                        st = st_all[:, 384 * kc : 384 * (kc + 1)]
                        tmp1 = sb.tile([128, 384], BF16, tag="tmp1", bufs=2, name=f"t1_{r}")
                        tmp2 = sb.tile([128, 384], BF16, tag="tmp2", bufs=2, name=f"t2_{r}")
                        qk = sb.tile([128, 384], BF16, tag="qk", bufs=4, name=f"qk_{r}")
                        nc.vector.tensor_tensor(tmp1[:], pqc[:, 0:384], ct, op=MULT)
                        pqv = pqc[:, 0:384].rearrange("p (h s j) -> p h s j", s=2, j=32)
                        t2v = tmp2[:].rearrange("p (h s j) -> p h s j", s=2, j=32)
                        stv = st.rearrange("p (h s j) -> p h s j", s=2, j=32)
                        # out real-half = q_i * (-sin); out imag-half = q_r * sin
                        nc.vector.tensor_tensor(
                            t2v[:, :, 0, :], pqv[:, :, 1, :], stv[:, :, 0, :], op=MULT
                        )
                        nc.vector.tensor_tensor(
                            t2v[:, :, 1, :], pqv[:, :, 0, :], stv[:, :, 1, :], op=MULT
                        )
                        nc.vector.tensor_tensor(qk[:], tmp1[:], tmp2[:], op=mybir.AluOpType.add)
                        # V -> bf16 into the ones-padded PV stationary
                        nc.scalar.copy(
                            vones[b][:, 128 * kc : 128 * kc + 64], pqc[:, 384:448]
                        )
                        # PE transposes, lagged one rowtile so the RoPE chain
                        # has a full projection's lead time
                        trans_pend.append((qk, b, kc, r))
                        if len(trans_pend) > 1:
                            do_transpose(trans_pend.pop(0))
                        yield
                while trans_pend:
                    do_transpose(trans_pend.pop(0))

            # ================= attention (per batch), ends with its AllToAll
            a2a_out = [None, None]

            def attn_stream(bb):
                a2a_in = dr.tile([2048, RPC], BF16, tag=f"a2ai{bb}", name=f"a2a_in{bb}")
                a2a_out[bb] = dr.tile(
                    [2048, RPC], BF16, tag=f"a2ao{bb}", name=f"a2a_out{bb}"
                )
                for hp in range(2):  # head pairs at partition halves 0:64/64:128
                    qtile = qt_sb[bb][hp]
                    for qc in range(4):  # 512-wide q chunks
                        jmax = 4 * qc + 3  # causal: 128-kpos chunks 0..jmax
                        ot = [
                            ps.tile([128, 512], F32, tag=f"ot{par}", bufs=1,
                                    name=f"ot_{bb}_{hp}_{qc}_{par}")
                            for par in range(2)
                        ]
                        pend = []  # lag-2 software pipeline for the PV matmuls

                        def do_pv(ent, bb=bb, jmax=jmax, ot=ot):
                            j, N, pt = ent
                            for par in range(2):
                                nc.tensor.matmul(
                                    ot[par][:, 512 - N : 512],
                                    vones[bb][:, 128 * j : 128 * (j + 1)],
                                    pt[:, 512 * par : 512 * par + N],
                                    start=(j == jmax),
                                    stop=(j == 0),
                                )

                        for j in range(jmax, -1, -1):
                            d = j - 4 * qc
                            # diagonal chunks: only q >= kpos columns are live
                            N = 512 - 128 * d if d >= 0 else 512
                            qlo = 512 * qc + (512 - N)
                            # both parities share one 2-bank psum tile so one
                            # wide exp (and one mask) covers the head pair
                            sp = ps.tile(
                                [128, 1024], F32, tag="sp", bufs=2,
                                name=f"s_{bb}_{hp}_{qc}_{j}",
                            )
                            for par in range(2):
                                r0 = 64 * par
                                nc.tensor.matmul(
                                    sp[:, 512 * par : 512 * par + N],
                                    kt_sb[bb][r0 : r0 + 64, 128 * j : 128 * (j + 1)],
                                    qtile[r0 : r0 + 64, qlo : qlo + N],
                                    start=True,
                                    stop=True,
                                )
                            pt = sb.tile(
                                [128, 1024], BF16, tag="pt", bufs=4,
                                name=f"pt_{bb}_{hp}_{qc}_{j}",
                            )
                            # the [N:512] gap holds garbage; never read below
                            nc.scalar.activation(
                                pt[:, 0 : 512 + N], sp[:, 0 : 512 + N], EXP, scale=0.125
                            )
                            if d >= 0:  # within-chunk causal mask (keep: q >= kpos)
                                nc.vector.tensor_tensor(
                                    pt[:, 0 : 512 + N], pt[:, 0 : 512 + N],
                                    maskm_sb[:, 0 : 512 + N], op=MULT,
                                )
                            pend.append((j, N, pt))
                            if len(pend) > 2:
                                do_pv(pend.pop(0))
                            yield
                        while pend:
                            do_pv(pend.pop(0))
                        # normalize: rows 0:64 are V^T P, rows 64:128 all hold
                        # the softmax sums (ones columns) -> pure-DVE norm
                        for par in range(2):
                            h = 2 * hp + par
                            sums = sb.tile([64, 512], F32, tag="sums", bufs=2,
                                           name=f"sm_{bb}_{h}_{qc}")
                            nc.vector.tensor_copy(sums[:], ot[par][64:128, :])
                            inv = sb.tile([64, 512], F32, tag="inv", bufs=2,
                                          name=f"iv_{bb}_{h}_{qc}")
                            nc.vector.reciprocal_approx_fast(inv[:], sums[:])
                            osb = sb.tile([64, 512], BF16, tag="osb", bufs=3,
                                          name=f"o_{bb}_{h}_{qc}")
                            nc.vector.tensor_tensor(
                                osb[:], ot[par][0:64, :], inv[:], op=MULT
                            )
                            # stage into AllToAll layout: dest j2 rows 256*j2..
                            for half in range(2):
                                j2 = 2 * qc + half
                                nc.sync.dma_start(
                                    out=a2a_in[
                                        256 * j2 + 64 * h : 256 * j2 + 64 * (h + 1), :
                                    ],
                                    in_=osb[:, 256 * half : 256 * (half + 1)],
                                )
                nc.gpsimd.collective_compute(
                    "AllToAll",
                    mybir.AluOpType.bypass,
                    replica_groups=[list(range(N_CORES))],
                    ins=[a2a_in[:].opt()],
                    outs=[a2a_out[bb][:].opt()],
                )

            # ================= output projection (my 512 rows @ wo)
            ats = [
                sb.tile([128, 512], BF16, tag=f"at{k}", name=f"at_{k}")
                for k in range(16)
            ]

            def oproj_stream(rows):
                for n in range(4):
                    for row in rows:
                        op = ps.tile([128, 512], F32, tag="pq", bufs=2,
                                     name=f"op_{n}_{row}")
                        for k in range(16):
                            nc.tensor.matmul(
                                op[:],
                                ats[k][:, 128 * row : 128 * (row + 1)],
                                wo_sb[k][:, 512 * n : 512 * (n + 1)],
                                start=(k == 0),
                                stop=(k == 15),
                            )
                            if k in (3, 7, 11):
                                yield
                        ob = sb.tile([128, 512], F32, tag="outsb", bufs=2,
                                     name=f"ob_{n}_{row}")
                        nc.vector.tensor_copy(ob[:], op[:])
                        nc.sync.dma_start(
                            out=out[128 * row : 128 * (row + 1), 512 * n : 512 * (n + 1)],
                            in_=ob[:],
                        )
                        yield

            # ---- phase 1: QKV b0
            for _ in qkv_stream(0):
                pass
            # ---- phase 2: attention b0 (ACT-bound) interleaved with QKV b1
            g_attn0, g_qkv1 = attn_stream(0), qkv_stream(1)
            cnt = 0
            for _ in g_attn0:
                cnt += 1
                if cnt % 5 != 0:
                    next(g_qkv1, None)
            for _ in g_qkv1:
                pass
            # at-tile b0 halves: gpsimd queue right behind the first AllToAll
            for k in range(16):
                nc.gpsimd.dma_start(
                    out=ats[k][:, 0:256], in_=a2a_out[0][128 * k : 128 * (k + 1), :]
                )
            # ---- phase 3: attention b1 interleaved with oproj phase A (b0 rows)
            g_attn1, g_opA = attn_stream(1), oproj_stream([0, 1])
            cnt = 0
            for _ in g_attn1:
                cnt += 1
                if cnt >= 48:
                    next(g_opA, None)
            for _ in g_opA:
                pass
            # at-tile b1 halves split across gpsimd + scalar (ACT idle by now)
            for k in range(16):
                eng = nc.gpsimd if k % 2 else nc.scalar
                eng.dma_start(
                    out=ats[k][:, 256:512], in_=a2a_out[1][128 * k : 128 * (k + 1), :]
                )
            # ---- phase 4: oproj phase B (b1 rows)
            for _ in oproj_stream([2, 3]):
                pass

    nc.finalize()
    return nc


_NC_CACHE = None


def _get_nc():
    global _NC_CACHE
    if _NC_CACHE is None:
        _NC_CACHE = build()
    return _NC_CACHE


def _prep_inputs(x, freqs_cis, mask, wq, wk, wv, wo):
    """Host-side sharding / layout prep. Returns per-core input maps."""
    bf16 = ml_dtypes.bfloat16
    xt = np.ascontiguousarray(x.reshape(ROWS, D).T.astype(bf16))  # [D, B*S]
    cos = np.ascontiguousarray(freqs_cis[:, :, 0])  # [S, 32]
    sin = np.ascontiguousarray(freqs_cis[:, :, 1])
    c64 = np.concatenate([cos, cos], axis=1)  # [S, 64]
    s64 = np.concatenate([-sin, sin], axis=1)
    ropec = np.ascontiguousarray(np.tile(c64, (1, 6)).astype(bf16))  # [S, 384]
    ropes = np.ascontiguousarray(np.tile(s64, (1, 6)).astype(bf16))
    # per-chunk 0/1 keep-mask (keep: q_local col >= kpos row), duplicated so
    # one DVE op can mask both head parities of a [e | o] pt tile
    tri = (np.arange(512)[None, :] >= np.arange(128)[:, None]).astype(bf16)
    maskm = np.ascontiguousarray(np.concatenate([tri, tri], axis=1))
    perm = np.concatenate([np.arange(0, 64, 2), np.arange(1, 64, 2)])  # de-interleave
    wo_c = np.ascontiguousarray(wo.astype(bf16))

    in_maps = []
    for c in range(N_CORES):
        heads = range(HPC * c, HPC * (c + 1))
        kv = c // 2
        wq_c = np.concatenate([wq[:, 64 * h + perm] for h in heads], axis=1)
        wk_c = wk[:, 64 * kv + perm]
        wv_c = wv[:, 64 * kv : 64 * (kv + 1)]
        wqkv_c = np.ascontiguousarray(
            np.concatenate([wq_c, wk_c, wk_c, wv_c], axis=1).astype(bf16)
        )
        in_maps.append(
            {
                "xt": xt,
                "wqkv": wqkv_c,
                "wo": wo_c,
                "ropec": ropec,
                "ropes": ropes,
                "maskm": maskm,
            }
        )
    return in_maps


def kernel(x, freqs_cis, mask, wq, wk, wv, wo, _trace=False, _trace_kwargs=None):
    nc = _get_nc()
    in_maps = _prep_inputs(
        np.asarray(x, np.float32),
        np.asarray(freqs_cis, np.float32),
        np.asarray(mask, np.float32),
        np.asarray(wq, np.float32),
        np.asarray(wk, np.float32),
        np.asarray(wv, np.float32),
        np.asarray(wo, np.float32),
    )
    kwargs = {}
    if _trace:
        kwargs["trace"] = True
        if _trace_kwargs:
            kwargs.update(_trace_kwargs)
    res = run_bass_kernel_spmd(nc, in_maps, core_ids=list(range(N_CORES)), **kwargs)
    full = np.empty((B, S, D), np.float32)
    for c in range(N_CORES):
        oc = res.results[c]["out"]
        full[0, RPC * c : RPC * (c + 1)] = oc[0:RPC]
        full[1, RPC * c : RPC * (c + 1)] = oc[RPC : 2 * RPC]
    if _trace:
        kernel.last_results = res
    return full


if __name__ == "__main__":
    print("building...")
    nc = _get_nc()
    print("built")
